# revision 3
# baseline (speedup 1.0000x reference)
"""GraphSAGE 2-layer kernel for 8 trn2 NeuronCores (Bass/Tile).

Strategy: shard the 20k layer-2 frontier nodes across 8 cores (2500 each).
Each core recomputes exactly the layer-1 hidden states its layer-2 shard
references (host-side dedup + remap, ~24k of 100k rows), so no collectives
are needed.  raw_features and the small weight matrices are replicated.

Per 128-node tile on device:
  - one indirect DMA gathers self + 10 neighbor rows (11 x 256 f32 per
    partition, node-major)
  - TensorEngine transposes the gathered chunks into feature-major PSUM,
    accumulating the 10 neighbor rows in-place (mean folded into W scale)
  - 4 accumulating matmuls against W.T produce the output tile in PSUM
  - DVE applies ReLU while evicting PSUM -> SBUF, then DMA to DRAM
"""

import sys

sys.path.insert(0, "/opt/trn_rl_repo")
sys.path.insert(0, "/opt/pypackages")

import numpy as np

# Model dims (fixed by the problem)
N_RAW, N1, N2, K, D, OUT = 200000, 100000, 20000, 10, 256, 256
N_CORES = 8
P = 128
W = K + 1  # rows gathered per node: self + K neighbors


def _build_program(T1: int, T2: int, n_rows1: int, n_rows2: int):
    """Bass program for one core.

    T1/T2: number of 128-node tiles for layer 1 / layer 2.
    n_rows1: rows of the raw feature table (N_RAW).
    n_rows2: rows of the local h1 table (T1*128).
    """
    import concourse.bacc as bacc
    import concourse.bass as bass
    import concourse.mybir as mybir
    import concourse.tile as tile
    from concourse.masks import make_identity

    f32 = mybir.dt.float32
    i32 = mybir.dt.int32

    nc = bacc.Bacc("TRN2", target_bir_lowering=False, debug=False)

    rawf = nc.dram_tensor("rawf", [n_rows1, D], f32, kind="ExternalInput")
    gidx1 = nc.dram_tensor("gidx1", [P, T1 * W], i32, kind="ExternalInput")
    gidx2 = nc.dram_tensor("gidx2", [P, T2 * W], i32, kind="ExternalInput")
    w1t = nc.dram_tensor("w1t", [2 * D, OUT], f32, kind="ExternalInput")
    w2t = nc.dram_tensor("w2t", [2 * OUT, OUT], f32, kind="ExternalInput")
    h1d = nc.dram_tensor("h1d", [n_rows2, OUT], f32)
    outd = nc.dram_tensor("out", [T2 * P, OUT], f32, kind="ExternalOutput")

    KC1 = (2 * D) // P   # contraction chunks layer 1 (4)
    KC2 = (2 * OUT) // P  # layer 2 (4)
    DC = D // P          # feature chunks per gathered row (2)

    with tile.TileContext(nc) as tc:
        with (
            tc.tile_pool(name="const", bufs=1) as cpool,
            tc.tile_pool(name="gbuf", bufs=4) as gpool,
            tc.tile_pool(name="xt", bufs=3) as xpool,
            tc.tile_pool(name="hout", bufs=4) as hpool,
            tc.tile_pool(name="pst", bufs=2, space="PSUM") as pspool,
            tc.tile_pool(name="pout", bufs=2, space="PSUM") as popool,
        ):
            ident = cpool.tile([P, P], f32)
            make_identity(nc, ident[:])

            idx1_sb = cpool.tile([P, T1 * W], i32)
            nc.sync.dma_start(idx1_sb[:], gidx1[:])
            idx2_sb = cpool.tile([P, T2 * W], i32)
            nc.sync.dma_start(idx2_sb[:], gidx2[:])

            w1_sb = cpool.tile([P, KC1 * OUT], f32)
            for c in range(KC1):
                nc.sync.dma_start(
                    w1_sb[:, c * OUT:(c + 1) * OUT], w1t[c * P:(c + 1) * P, :]
                )
            w2_sb = cpool.tile([P, KC2 * OUT], f32)
            for c in range(KC2):
                nc.sync.dma_start(
                    w2_sb[:, c * OUT:(c + 1) * OUT], w2t[c * P:(c + 1) * P, :]
                )

            def layer(T, idx_sb, table, w_sb, kc, dst):
                for i in range(T):
                    g = gpool.tile([P, W * D], f32, tag="g")
                    # HW indirect DMA honors one index per partition, so
                    # gather the 11 rows per node as 11 single-index ops
                    # writing disjoint column slices of the same tile.
                    for k in range(W):
                        nc.gpsimd.indirect_dma_start(
                            out=g[:, k * D:(k + 1) * D],
                            out_offset=None,
                            in_=table[:],
                            in_offset=bass.IndirectOffsetOnAxis(
                                ap=idx_sb[:, i * W + k:i * W + k + 1], axis=0
                            ),
                        )
                    # feature-major combined = [self ; sum_k neigh_k]
                    psT = pspool.tile([P, 2 * D], f32, tag="psT")
                    for c in range(DC):
                        nc.tensor.transpose(
                            psT[:, c * P:(c + 1) * P],
                            g[:, c * P:(c + 1) * P],
                            ident[:],
                        )
                    for c in range(DC):
                        o = D + c * P
                        for k in range(K):
                            col = (1 + k) * D + c * P
                            nc.tensor.matmul(
                                out=psT[:, o:o + P],
                                lhsT=g[:, col:col + P],
                                rhs=ident[:],
                                is_transpose=True,
                                start=(k == 0),
                                stop=(k == K - 1),
                            )
                    xT = xpool.tile([P, 2 * D], f32, tag="xT")
                    nc.vector.tensor_copy(xT[:], psT[:])
                    ph = popool.tile([P, OUT], f32, tag="ph")
                    for c in range(kc):
                        nc.tensor.matmul(
                            out=ph[:],
                            lhsT=xT[:, c * P:(c + 1) * P],
                            rhs=w_sb[:, c * OUT:(c + 1) * OUT],
                            start=(c == 0),
                            stop=(c == kc - 1),
                        )
                    h = hpool.tile([P, OUT], f32, tag="h")
                    nc.vector.tensor_scalar_max(h[:], ph[:], 0.0)
                    nc.sync.dma_start(dst[i * P:(i + 1) * P, :], h[:])

            layer(T1, idx1_sb, rawf, w1_sb, KC1, h1d)
            layer(T2, idx2_sb, h1d, w2_sb, KC2, outd)

    nc.compile()
    return nc


def _prep_core(c, nodes1, neigh1, self2, neigh2, T2):
    """Host-side index prep for core c. Returns (U, gidx-builder inputs)."""
    n2_lo, n2_hi = c * (N2 // N_CORES), (c + 1) * (N2 // N_CORES)
    s2 = self2[n2_lo:n2_hi]
    ng2 = neigh2[n2_lo:n2_hi]
    needed = np.unique(np.concatenate([s2, ng2.ravel()]))
    s2l = np.searchsorted(needed, s2).astype(np.int32)
    ng2l = np.searchsorted(needed, ng2).astype(np.int32)
    U = needed.size

    # layer-1 gather rows for this core's needed nodes
    l1 = np.empty((U, W), dtype=np.int32)
    l1[:, 0] = nodes1[needed]
    l1[:, 1:] = neigh1[needed]

    l2 = np.zeros((T2 * P, W), dtype=np.int32)
    l2[: s2l.size, 0] = s2l
    l2[: ng2l.shape[0], 1:] = ng2l
    return U, l1, l2


def _swizzle(idx_rows, T):
    """[T*P, W] row-major -> [P, T*W] so tile i's idx = [:, i*W:(i+1)*W]."""
    return (
        idx_rows.reshape(T, P, W).transpose(1, 0, 2).reshape(P, T * W).copy()
    )


def kernel(raw_features, W1, W2, nodes1, neigh1, self2, neigh2, _trace=False):
    from concourse.bass_utils import run_bass_kernel_spmd

    raw_features = np.ascontiguousarray(raw_features, dtype=np.float32)
    W1 = np.asarray(W1, dtype=np.float32)
    W2 = np.asarray(W2, dtype=np.float32)
    nodes1 = np.asarray(nodes1, dtype=np.int32)
    neigh1 = np.asarray(neigh1, dtype=np.int32)
    self2 = np.asarray(self2, dtype=np.int32)
    neigh2 = np.asarray(neigh2, dtype=np.int32)

    T2 = -(-(N2 // N_CORES) // P)  # 20 tiles of 128 (2560 >= 2500)

    preps = [
        _prep_core(c, nodes1, neigh1, self2, neigh2, T2) for c in range(N_CORES)
    ]
    T1 = max(-(-u // P) for u, _, _ in preps)

    # fold the 1/K neighbor-mean into the W columns that touch the agg half
    w1t = np.concatenate([W1[:, :D], W1[:, D:] / K], axis=1).T.copy()
    w2t = np.concatenate([W2[:, :OUT], W2[:, OUT:] / K], axis=1).T.copy()

    in_maps = []
    for c in range(N_CORES):
        U, l1, l2 = preps[c]
        l1p = np.zeros((T1 * P, W), dtype=np.int32)
        l1p[:U] = l1
        in_maps.append(
            {
                "rawf": raw_features,
                "gidx1": _swizzle(l1p, T1),
                "gidx2": _swizzle(l2, T2),
                "w1t": w1t,
                "w2t": w2t,
            }
        )

    nc = _build_program(T1, T2, N_RAW, T1 * P)
    res = run_bass_kernel_spmd(
        nc, in_maps, list(range(N_CORES)), trace=_trace
    )

    n_per = N2 // N_CORES
    out = np.concatenate(
        [res.results[c]["out"][:n_per] for c in range(N_CORES)], axis=0
    )
    if _trace:
        return out, res
    return out


# revision 5
# speedup vs baseline: 45.6562x; 45.6562x over previous
"""GraphSAGE 2-layer kernel for 8 trn2 NeuronCores (Bass/Tile).

Strategy: shard the 20k layer-2 frontier nodes across 8 cores (2500 each).
Each core recomputes exactly the layer-1 hidden states its layer-2 shard
references (host-side dedup + remap, ~24k of 100k rows), so no collectives
are needed.  raw_features and the small weight matrices are replicated.

Per 128-node tile on device:
  - one indirect DMA gathers self + 10 neighbor rows (11 x 256 f32 per
    partition, node-major)
  - TensorEngine transposes the gathered chunks into feature-major PSUM,
    accumulating the 10 neighbor rows in-place (mean folded into W scale)
  - 4 accumulating matmuls against W.T produce the output tile in PSUM
  - DVE applies ReLU while evicting PSUM -> SBUF, then DMA to DRAM
"""

import sys

sys.path.insert(0, "/opt/trn_rl_repo")
sys.path.insert(0, "/opt/pypackages")

import numpy as np

# Model dims (fixed by the problem)
N_RAW, N1, N2, K, D, OUT = 200000, 100000, 20000, 10, 256, 256
N_CORES = 8
P = 128
W = K + 1  # rows gathered per node: self + K neighbors


def _build_program(T1: int, T2: int, n_rows1: int, n_rows2: int):
    """Bass program for one core.

    T1/T2: number of 128-node tiles for layer 1 / layer 2.
    n_rows1: rows of the raw feature table (N_RAW).
    n_rows2: rows of the local h1 table (T1*128).
    """
    import concourse.bacc as bacc
    import concourse.bass as bass
    import concourse.mybir as mybir
    import concourse.tile as tile
    from concourse.masks import make_identity

    f32 = mybir.dt.float32
    i32 = mybir.dt.int32

    nc = bacc.Bacc("TRN2", target_bir_lowering=False, debug=False)

    rawf = nc.dram_tensor("rawf", [n_rows1, D], f32, kind="ExternalInput")
    gidx1 = nc.dram_tensor("gidx1", [P, T1 * W], i32, kind="ExternalInput")
    gidx2 = nc.dram_tensor("gidx2", [P, T2 * W], i32, kind="ExternalInput")
    w1t = nc.dram_tensor("w1t", [2 * D, OUT], f32, kind="ExternalInput")
    w2t = nc.dram_tensor("w2t", [2 * OUT, OUT], f32, kind="ExternalInput")
    h1d = nc.dram_tensor("h1d", [n_rows2, OUT], f32)
    outd = nc.dram_tensor("out", [T2 * P, OUT], f32, kind="ExternalOutput")

    KC1 = (2 * D) // P   # contraction chunks layer 1 (4)
    KC2 = (2 * OUT) // P  # layer 2 (4)
    DC = D // P          # feature chunks per gathered row (2)

    with tile.TileContext(nc) as tc:
        with (
            tc.tile_pool(name="const", bufs=1) as cpool,
            tc.tile_pool(name="gbuf", bufs=24) as gpool,
            tc.tile_pool(name="xt", bufs=3) as xpool,
            tc.tile_pool(name="hout", bufs=4) as hpool,
            tc.tile_pool(name="pst", bufs=2, space="PSUM") as pspool,
            tc.tile_pool(name="pout", bufs=2, space="PSUM") as popool,
        ):
            ident = cpool.tile([P, P], f32)
            make_identity(nc, ident[:])

            idx1_sb = cpool.tile([P, T1 * W], i32)
            nc.sync.dma_start(idx1_sb[:], gidx1[:])
            idx2_sb = cpool.tile([P, T2 * W], i32)
            nc.sync.dma_start(idx2_sb[:], gidx2[:])

            w1_sb = cpool.tile([P, KC1 * OUT], f32)
            for c in range(KC1):
                nc.sync.dma_start(
                    w1_sb[:, c * OUT:(c + 1) * OUT], w1t[c * P:(c + 1) * P, :]
                )
            w2_sb = cpool.tile([P, KC2 * OUT], f32)
            for c in range(KC2):
                nc.sync.dma_start(
                    w2_sb[:, c * OUT:(c + 1) * OUT], w2t[c * P:(c + 1) * P, :]
                )

            def layer(T, idx_sb, table, w_sb, kc, dst):
                for i in range(T):
                    # HW indirect DMA honors one index per partition, so
                    # gather the 11 rows per node as 11 single-index ops.
                    # Each op gets its own pool buffer: sharing one tile
                    # serializes the DMAs (~40us/tile vs ~2.5us/op).
                    gs = []
                    for k in range(W):
                        gk = gpool.tile([P, D], f32, tag="g")
                        nc.gpsimd.indirect_dma_start(
                            out=gk[:],
                            out_offset=None,
                            in_=table[:],
                            in_offset=bass.IndirectOffsetOnAxis(
                                ap=idx_sb[:, i * W + k:i * W + k + 1], axis=0
                            ),
                        )
                        gs.append(gk)
                    # feature-major combined = [self ; sum_k neigh_k]
                    psT = pspool.tile([P, 2 * D], f32, tag="psT")
                    for c in range(DC):
                        nc.tensor.transpose(
                            psT[:, c * P:(c + 1) * P],
                            gs[0][:, c * P:(c + 1) * P],
                            ident[:],
                        )
                    for c in range(DC):
                        o = D + c * P
                        for k in range(K):
                            nc.tensor.matmul(
                                out=psT[:, o:o + P],
                                lhsT=gs[1 + k][:, c * P:(c + 1) * P],
                                rhs=ident[:],
                                is_transpose=True,
                                start=(k == 0),
                                stop=(k == K - 1),
                            )
                    xT = xpool.tile([P, 2 * D], f32, tag="xT")
                    nc.vector.tensor_copy(xT[:], psT[:])
                    ph = popool.tile([P, OUT], f32, tag="ph")
                    for c in range(kc):
                        nc.tensor.matmul(
                            out=ph[:],
                            lhsT=xT[:, c * P:(c + 1) * P],
                            rhs=w_sb[:, c * OUT:(c + 1) * OUT],
                            start=(c == 0),
                            stop=(c == kc - 1),
                        )
                    h = hpool.tile([P, OUT], f32, tag="h")
                    nc.vector.tensor_scalar_max(h[:], ph[:], 0.0)
                    nc.sync.dma_start(dst[i * P:(i + 1) * P, :], h[:])

            layer(T1, idx1_sb, rawf, w1_sb, KC1, h1d)
            layer(T2, idx2_sb, h1d, w2_sb, KC2, outd)

    nc.compile()
    return nc


def _prep_core(c, nodes1, neigh1, self2, neigh2, T2):
    """Host-side index prep for core c. Returns (U, gidx-builder inputs)."""
    n2_lo, n2_hi = c * (N2 // N_CORES), (c + 1) * (N2 // N_CORES)
    s2 = self2[n2_lo:n2_hi]
    ng2 = neigh2[n2_lo:n2_hi]
    needed = np.unique(np.concatenate([s2, ng2.ravel()]))
    s2l = np.searchsorted(needed, s2).astype(np.int32)
    ng2l = np.searchsorted(needed, ng2).astype(np.int32)
    U = needed.size

    # layer-1 gather rows for this core's needed nodes
    l1 = np.empty((U, W), dtype=np.int32)
    l1[:, 0] = nodes1[needed]
    l1[:, 1:] = neigh1[needed]

    l2 = np.zeros((T2 * P, W), dtype=np.int32)
    l2[: s2l.size, 0] = s2l
    l2[: ng2l.shape[0], 1:] = ng2l
    return U, l1, l2


def _swizzle(idx_rows, T):
    """[T*P, W] row-major -> [P, T*W] so tile i's idx = [:, i*W:(i+1)*W]."""
    return (
        idx_rows.reshape(T, P, W).transpose(1, 0, 2).reshape(P, T * W).copy()
    )


def kernel(raw_features, W1, W2, nodes1, neigh1, self2, neigh2, _trace=False):
    from concourse.bass_utils import run_bass_kernel_spmd

    raw_features = np.ascontiguousarray(raw_features, dtype=np.float32)
    W1 = np.asarray(W1, dtype=np.float32)
    W2 = np.asarray(W2, dtype=np.float32)
    nodes1 = np.asarray(nodes1, dtype=np.int32)
    neigh1 = np.asarray(neigh1, dtype=np.int32)
    self2 = np.asarray(self2, dtype=np.int32)
    neigh2 = np.asarray(neigh2, dtype=np.int32)

    T2 = -(-(N2 // N_CORES) // P)  # 20 tiles of 128 (2560 >= 2500)

    preps = [
        _prep_core(c, nodes1, neigh1, self2, neigh2, T2) for c in range(N_CORES)
    ]
    T1 = max(-(-u // P) for u, _, _ in preps)

    # fold the 1/K neighbor-mean into the W columns that touch the agg half
    w1t = np.concatenate([W1[:, :D], W1[:, D:] / K], axis=1).T.copy()
    w2t = np.concatenate([W2[:, :OUT], W2[:, OUT:] / K], axis=1).T.copy()

    in_maps = []
    for c in range(N_CORES):
        U, l1, l2 = preps[c]
        l1p = np.zeros((T1 * P, W), dtype=np.int32)
        l1p[:U] = l1
        in_maps.append(
            {
                "rawf": raw_features,
                "gidx1": _swizzle(l1p, T1),
                "gidx2": _swizzle(l2, T2),
                "w1t": w1t,
                "w2t": w2t,
            }
        )

    nc = _build_program(T1, T2, N_RAW, T1 * P)
    res = run_bass_kernel_spmd(
        nc, in_maps, list(range(N_CORES)), trace=_trace
    )

    n_per = N2 // N_CORES
    out = np.concatenate(
        [res.results[c]["out"][:n_per] for c in range(N_CORES)], axis=0
    )
    if _trace:
        return out, res
    return out
